# revision 19
# baseline (speedup 1.0000x reference)
"""Trainium2 Bass kernel for nn_DASAttentionGate (depthwise-sep conv -> InstanceNorm
-> ReLU -> offset conv -> deformable conv -> GroupNorm -> sigmoid gate).

Sharding: 8 cores = 4 samples x 2 H-halves (48 output rows each). Cross-core
communication: two tiny AllReduces (InstanceNorm + GroupNorm statistics) within
sample pairs.

Deformable conv strategy ("quad gather"):
  - h_n (normalized activations) transposed to pixel-major h_T and written to a
    DRAM table z4 of 2x2 pixel quads: z4[(y,x)] = [h(y,x), h(y,x+1), h(y+1,x),
    h(y+1,x+1)] for all 128 channels (bf16, 1KB per block).
  - Offsets -> per (tap, pixel) a single int16 block index + 4 bilinear corner
    weights (device-computed, staged through DRAM to re-tile into the gather's
    pixel-mod-128 layout).
  - gpsimd.dma_gather fetches one 1KB quad per (tap, pixel).
  - DVE scales the 4 corners by their weights (bf16 2x mode via duplicated-mask
    APs), then PE sums corners + transposes back to channel-major via 4
    accumulating identity-RHS matmuls, and finally contracts channels with the
    deform weights (einsum), accumulating all 9 taps in PSUM.
"""

import os
import sys

for _p in ("/opt/trn_rl_repo",):
    if os.path.isdir(_p) and _p not in sys.path:
        sys.path.insert(0, _p)

import numpy as np
import ml_dtypes

import concourse.bass as bass
import concourse.bacc as bacc
import concourse.tile as tile
from concourse import mybir
from concourse.bass_utils import run_bass_kernel_spmd

F32 = mybir.dt.float32
F32R = mybir.dt.float32r
BF16 = mybir.dt.bfloat16
I16 = mybir.dt.int16
I32 = mybir.dt.int32
AF = mybir.ActivationFunctionType
OP = mybir.AluOpType

# problem geometry (hardcoded per the task contract)
B, C, H, W = 4, 128, 96, 96
NCORES = 8
RPC = 48           # output rows per core
GR, GC = 64, 112   # h grid: rows r0-8..r0+55, cols -8..103
XR, XC = 66, 114   # x grid: rows r0-9..r0+56, cols -9..104
ZY, ZX = 63, 112   # quad block grid: y0' in 0..62, x0' in 0..110 (stride ZX)
NB = ZY * ZX       # 7056 blocks
NPIX = RPC * W     # 4608 output pixels per core
NT = NPIX // 128   # 36 pixel tiles
BLKT = 9           # pixel tiles per gather block
NBLK = NT // BLKT  # 3 gather blocks
NIDX = BLKT * 128  # 1536 indices per gather call
EPS = 1e-5

_CACHE = {}


def _build_program():
    nc = bacc.Bacc("TRN2", target_bir_lowering=False, debug=False,
                   num_devices=NCORES)

    # ---- I/O ----
    x_d = nc.dram_tensor("x_sh", [C, XR, XC], F32, kind="ExternalInput")
    vm_d = nc.dram_tensor("vrow", [C, GR], BF16, kind="ExternalInput")
    wf_d = nc.dram_tensor("wf", [C, 9, C], F32, kind="ExternalInput")
    b1_d = nc.dram_tensor("b1c", [C, 1], F32, kind="ExternalInput")
    ow_d = nc.dram_tensor("ow", [C, 9, 18], BF16, kind="ExternalInput")
    ob_d = nc.dram_tensor("obr", [128, 18], F32, kind="ExternalInput")
    wd_d = nc.dram_tensor("wd", [C, 9, C], BF16, kind="ExternalInput")
    db_d = nc.dram_tensor("dbc", [C, 1], F32, kind="ExternalInput")
    gw_d = nc.dram_tensor("gwc", [C, 1], F32, kind="ExternalInput")
    gb_d = nc.dram_tensor("gbc", [C, 1], F32, kind="ExternalInput")
    id_d = nc.dram_tensor("idn", [128, 128], BF16, kind="ExternalInput")
    on_d = nc.dram_tensor("onec", [C, 1], F32, kind="ExternalInput")
    io_d = nc.dram_tensor("iotc", [96, RPC, 9], F32, kind="ExternalInput")
    out_d = nc.dram_tensor("out_sh", [C, RPC, W], F32, kind="ExternalOutput")

    groups = [[0, 1], [2, 3], [4, 5], [6, 7]]

    with tile.TileContext(nc) as tc:
        with (
            tc.tile_pool(name="const", bufs=1) as constp,
            tc.tile_pool(name="xbuf", bufs=1) as xpool,
            tc.tile_pool(name="hbuf", bufs=1) as hpool,
            tc.tile_pool(name="mwork", bufs=1) as mpool,
            tc.tile_pool(name="gbuf", bufs=1) as gpool,
            tc.tile_pool(name="sbig", bufs=1) as spool,
            tc.tile_pool(name="ps", bufs=6, space="PSUM") as psp,
            tc.tile_pool(name="dram", bufs=1, space="DRAM") as dramp,
        ):
            # ---- load constants ----
            wf = constp.tile([C, 9, C], F32R, tag="wf")
            nc.sync.dma_start(wf[:], wf_d[:].bitcast(F32R))
            b1 = constp.tile([C, 1], F32, tag="b1")
            nc.sync.dma_start(b1[:], b1_d[:])
            ow = constp.tile([C, 9, 18], BF16, tag="ow")
            nc.sync.dma_start(ow[:], ow_d[:])
            ob = constp.tile([128, 18], F32, tag="ob")
            nc.sync.dma_start(ob[:], ob_d[:])
            wd = constp.tile([C, 9, C], BF16, tag="wd")
            nc.sync.dma_start(wd[:], wd_d[:])
            db = constp.tile([C, 1], F32, tag="db")
            nc.sync.dma_start(db[:], db_d[:])
            gw = constp.tile([C, 1], F32, tag="gw")
            nc.sync.dma_start(gw[:], gw_d[:])
            gb = constp.tile([C, 1], F32, tag="gb")
            nc.sync.dma_start(gb[:], gb_d[:])
            idn = constp.tile([128, 128], BF16, tag="idn")
            nc.sync.dma_start(idn[:], id_d[:])
            onec = constp.tile([C, 1], F32, tag="onec")
            nc.sync.dma_start(onec[:], on_d[:])
            vm = constp.tile([C, GR], BF16, tag="vm")
            nc.sync.dma_start(vm[:], vm_d[:])

            # ---- conv1 (fused depthwise+pointwise, f32r) ----
            xs = xpool.tile([C, XR, XC], F32R, tag="xs")
            nc.sync.dma_start(xs[:, 0:33, :], x_d[:, 0:33, :].bitcast(F32R))
            nc.sync.dma_start(xs[:, 33:66, :], x_d[:, 33:66, :].bitcast(F32R))

            hraw = hpool.tile([C, GR, GC], F32, tag="hraw")
            CH = 4  # grid rows per psum chunk
            for ch in range(GR // CH):
                gr0 = ch * CH
                pt = psp.tile([128, CH * GC], F32, tag="ps")
                for t in range(9):
                    ty, tx = t // 3, t % 3
                    rhs = xs[:, gr0 + ty:gr0 + ty + CH, tx:tx + GC]
                    nc.tensor.matmul(
                        pt[:], wf[:, t, :], rhs,
                        start=(t == 0), stop=(t == 8))
                nc.scalar.activation(
                    hraw[:, gr0:gr0 + CH, :].rearrange("p a b -> p (a b)"),
                    pt[:], AF.Identity, bias=b1[:])

            # ---- InstanceNorm stats over own 48 valid rows ----
            valid = hraw[:, 8:56, 8:104]
            st = mpool.tile([C, 2], F32, tag="st")
            nc.vector.tensor_reduce(st[:, 0:1], valid, mybir.AxisListType.XY,
                                    OP.add)
            sq = spool.tile([C, NPIX], F32, tag="big")
            nc.scalar.activation(sq[:].rearrange("p (a b) -> p a b", a=RPC),
                                 valid, AF.Square, accum_out=st[:, 1:2])

            cc_in = dramp.tile([C, 2], F32, tag="cci")
            cc_out = dramp.tile([C, 2], F32, tag="cco")
            nc.sync.dma_start(cc_in[:], st[:])
            nc.gpsimd.collective_compute(
                "AllReduce", OP.add, replica_groups=groups,
                ins=[cc_in[:].opt()], outs=[cc_out[:].opt()])
            stg = mpool.tile([C, 2], F32, tag="stg")
            nc.sync.dma_start(stg[:], cc_out[:])

            # mean/rstd per channel
            mom = mpool.tile([C, 2], F32, tag="mom")
            nc.vector.tensor_scalar(mom[:], stg[:], 1.0 / (H * W), None,
                                    OP.mult)
            var = mpool.tile([C, 1], F32, tag="var")
            nc.vector.tensor_tensor(var[:], mom[:, 0:1], mom[:, 0:1], OP.mult)
            nc.vector.tensor_tensor(var[:], mom[:, 1:2], var[:], OP.subtract)
            nc.vector.tensor_scalar(var[:], var[:], EPS, None, OP.add)
            rstd = mpool.tile([C, 1], F32, tag="rstd")
            nc.scalar.activation(rstd[:], var[:], AF.Sqrt)
            nc.vector.reciprocal(rstd[:], rstd[:])
            nbias = mpool.tile([C, 1], F32, tag="nbias")
            nc.vector.tensor_tensor(nbias[:], mom[:, 0:1], rstd[:], OP.mult)
            nc.vector.tensor_scalar(nbias[:], nbias[:], -1.0, None, OP.mult)

            # ---- h_n (bf16, masked) + f32 shortcut ----
            hn = hpool.tile([C, GR, GC], BF16, tag="hn")
            nc.scalar.activation(hn[:], hraw[:], AF.Relu, bias=nbias[:],
                                 scale=rstd[:])
            vmb = vm[:].unsqueeze(2).broadcast_to((C, GR, GC))
            nc.vector.tensor_tensor(hn[:], hn[:], vmb, OP.mult)
            nc.gpsimd.memset(hn[:, :, 0:8], 0.0)
            nc.gpsimd.memset(hn[:, :, 104:112], 0.0)
            short = spool.tile([C, NPIX], F32, tag="short")
            nc.scalar.activation(
                short[:].rearrange("p (a b) -> p a b", a=RPC),
                hraw[:, 8:56, 8:104], AF.Relu, bias=nbias[:], scale=rstd[:])

            # ---- h_T (pixel-major transpose of h_n rows) ----
            hT = hpool.tile([112, GR, 128], BF16, tag="hraw")
            for gr in range(GR):
                pt = psp.tile([112, 128], F32, tag="ps")
                nc.tensor.matmul(pt[:], hn[:, gr, :], idn[:],
                                 start=True, stop=True)
                nc.scalar.activation(hT[:, gr, :], pt[:], AF.Copy)

            # ---- z4 quad table in DRAM ----
            z4 = dramp.tile([NB, 512], BF16, tag="z4")
            z4v = z4[:].rearrange("(y x) (j c) -> x y j c", x=ZX, j=4)
            for j, (jy, jx) in enumerate(((0, 0), (0, 1), (1, 0), (1, 1))):
                nc.sync.dma_start(z4v[0:111, :, j, :],
                                  hT[jx:jx + 111, jy:jy + ZY, :])

            # ---- offset conv (bf16, output transposed per row) ----
            offT = mpool.tile([96, RPC, 18], F32, tag="offT")
            for r in range(RPC):
                gr = r + 8
                po = psp.tile([96, 18], F32, tag="ps")
                for t in range(9):
                    ty, tx = t // 3, t % 3
                    lhsT = hn[:, gr + ty - 1, 7 + tx:7 + tx + 96]
                    nc.tensor.matmul(po[:], lhsT, ow[:, t, :],
                                     start=(t == 0), stop=(t == 8))
                nc.scalar.activation(offT[:, r, :], po[:], AF.Copy)
            obv = ob[0:96, :].unsqueeze(1).broadcast_to((96, RPC, 18))
            nc.vector.tensor_tensor(offT[:], offT[:], obv, OP.add)

            # ---- bilinear masks + gather indices ----
            # all [96, RPC, 9] f32 grids (partition = w)
            def mk(tag):
                return mpool.tile([96, RPC, 9], F32, tag=tag, name=tag)

            offv = offT[:].rearrange("p r (k two) -> p r k two", two=2)
            oy, ox = offv[:, :, :, 0], offv[:, :, :, 1]
            it32 = mpool.tile([96, RPC, 9], I32, tag="it32")
            kf = mk("kf")
            gt = mk("gt")
            fy = mk("fy")
            ly = mk("ly")
            fx = mk("fx")
            lx = mk("lx")
            for (o_, f_, l_) in ((oy, fy, ly), (ox, fx, lx)):
                nc.vector.tensor_copy(it32[:], o_)
                nc.vector.tensor_copy(kf[:], it32[:])
                nc.vector.tensor_tensor(gt[:], kf[:], o_, OP.is_gt)
                nc.vector.tensor_tensor(f_[:], kf[:], gt[:], OP.subtract)
                nc.vector.tensor_tensor(l_[:], o_, f_[:], OP.subtract)
            uy = mk("uy")
            ux = mk("ux")
            nc.vector.tensor_scalar(uy[:], ly[:], -1.0, 1.0, OP.mult, OP.add)
            nc.vector.tensor_scalar(ux[:], lx[:], -1.0, 1.0, OP.mult, OP.add)
            a_sb = mpool.tile([96, RPC, 9, 4, 2], BF16, tag="a_sb")

            def dup2(ap):
                return ap.unsqueeze(3).broadcast_to((96, RPC, 9, 2))

            for j, (fa, fb) in enumerate(((uy, ux), (uy, lx),
                                          (ly, ux), (ly, lx))):
                nc.vector.tensor_tensor(a_sb[:, :, :, j, :],
                                        dup2(fa[:]), dup2(fb[:]), OP.mult)

            # idx = iota + 112*fy + fx, clamped to [0, NB-1]
            iot = mpool.tile([96, RPC, 9], F32, tag="iot")
            nc.sync.dma_start(iot[:], io_d[:])
            idxf = mk("idxf")
            nc.vector.tensor_scalar(idxf[:], fy[:], float(ZX), None, OP.mult)
            nc.vector.tensor_tensor(idxf[:], idxf[:], fx[:], OP.add)
            nc.vector.tensor_tensor(idxf[:], idxf[:], iot[:], OP.add)
            nc.vector.tensor_scalar(idxf[:], idxf[:], 0.0, float(NB - 1),
                                    OP.max, OP.min)
            idx_sb = mpool.tile([96, RPC, 9], I16, tag="idx_sb")
            nc.vector.tensor_copy(idx_sb[:], idxf[:])

            # ---- stage masks/indices through DRAM to re-tile ----
            a_d = dramp.tile([9, NPIX, 8], BF16, tag="a_d")
            for kk in range(9):
                nc.sync.dma_start(
                    a_d[kk].rearrange("(r w) j -> w r j", w=96),
                    a_sb[:, :, kk, :, :].rearrange("p r j t -> p r (j t)"))
            idx_d = dramp.tile([9, NPIX], I16, tag="idx_d")
            for kk in range(9):
                nc.sync.dma_start(
                    idx_d[kk].rearrange("(r w) -> w r", w=96),
                    idx_sb[:, :, kk])

            # wrapped-16 gather index tiles: [128, 27 (kk,blk), NIDX/16]
            NC16 = NIDX // 16
            idx_w = gpool.tile([128, 9 * NBLK, NC16], I16, tag="idx_w")
            for kk in range(9):
                for blki in range(NBLK):
                    isrc = idx_d[kk][blki * NIDX:(blki + 1) * NIDX]
                    nc.sync.dma_start(
                        idx_w[0:16, kk * NBLK + blki, :],
                        isrc.rearrange("(col p) -> p col", p=16))
            for g16 in range(1, 8):
                nc.sync.dma_start(idx_w[g16 * 16:(g16 + 1) * 16, :, :],
                                  idx_w[0:16, :, :])
            # corner weights, duplicated pairs: [128, 9, NT, 4, 2]
            a_w = gpool.tile([128, 9, NT, 4, 2], BF16, tag="a_w")
            for kk in range(9):
                nc.sync.dma_start(
                    a_w[:, kk, :, :, :].rearrange("p g j t -> p g (j t)"),
                    a_d[kk].rearrange("(g p) j -> p g j", p=128))

            # ---- gather + scale + corner-sum/transpose + einsum ----
            d_sb = spool.tile([C, NT, 128], F32, tag="dsb")
            for blk in range(NBLK):
                sampT = xpool.tile([128, 9, BLKT, 128], BF16, tag="xs")
                for kk in range(9):
                    g_t = gpool.tile([128, BLKT, 4, 128], BF16, tag="g_t", bufs=2)
                    nc.gpsimd.dma_gather(
                        g_t[:].rearrange("p a b c -> p a (b c)"),
                        z4[:], idx_w[:, kk * NBLK + blk, :],
                        NIDX, NIDX, 512, queue_num=0,
                        single_packet=False)
                    # scale corners by bilinear weights (bf16 2x via dup pairs)
                    gv = g_t[:].rearrange("p a b (c two) -> p a b c two",
                                          two=2)
                    for j in range(4):
                        av = a_w[:, kk, blk * BLKT:(blk + 1) * BLKT, j, :]
                        av = av.unsqueeze(2).broadcast_to((128, BLKT, 64, 2))
                        nc.vector.tensor_tensor(gv[:, :, j], gv[:, :, j], av,
                                                OP.mult)
                    # sum 4 corners + transpose to channel-major via PE
                    for t in range(BLKT):
                        pt = psp.tile([128, 128], F32, tag="ps")
                        for j in range(4):
                            nc.tensor.matmul(pt[:], g_t[:, t, j, :], idn[:],
                                             start=(j == 0), stop=(j == 3))
                        nc.scalar.activation(sampT[:, kk, t, :], pt[:],
                                             AF.Copy)
                # einsum: accumulate 9 taps
                for t in range(BLKT):
                    pd = psp.tile([128, 128], F32, tag="ps")
                    for kk in range(9):
                        nc.tensor.matmul(pd[:], wd[:, kk, :], sampT[:, kk, t, :],
                                         start=(kk == 0), stop=(kk == 8))
                    nc.scalar.activation(d_sb[:, blk * BLKT + t, :], pd[:],
                                         AF.Identity, bias=db[:])

            # ---- GroupNorm stats (whole sample) ----
            gst = mpool.tile([C, 2], F32, tag="gst")
            nc.vector.tensor_reduce(gst[:, 0:1], d_sb[:],
                                    mybir.AxisListType.XY, OP.add)
            nc.scalar.activation(sq[:].rearrange("p (a b) -> p a b", a=NT),
                                 d_sb[:], AF.Square, accum_out=gst[:, 1:2])
            pg = psp.tile([1, 2], F32, tag="ps")
            nc.tensor.matmul(pg[:], onec[:], gst[:], start=True, stop=True)
            gred = mpool.tile([1, 2], F32, tag="gred")
            nc.scalar.activation(gred[:], pg[:], AF.Copy)
            ccg_in = dramp.tile([1, 2], F32, tag="ccgi")
            ccg_out = dramp.tile([1, 2], F32, tag="ccgo")
            nc.sync.dma_start(ccg_in[:], gred[:])
            nc.gpsimd.collective_compute(
                "AllReduce", OP.add, replica_groups=groups,
                ins=[ccg_in[:].opt()], outs=[ccg_out[:].opt()])
            gsc = mpool.tile([1, 2], F32, tag="gsc")
            nc.sync.dma_start(gsc[:], ccg_out[:])
            gall = mpool.tile([128, 2], F32, tag="gall")
            nc.gpsimd.partition_broadcast(gall[:], gsc[:], 128)

            gmom = mpool.tile([C, 2], F32, tag="gmom")
            nc.vector.tensor_scalar(gmom[:], gall[:], 1.0 / (C * H * W), None,
                                    OP.mult)
            gvar = mpool.tile([C, 1], F32, tag="gvar")
            nc.vector.tensor_tensor(gvar[:], gmom[:, 0:1], gmom[:, 0:1],
                                    OP.mult)
            nc.vector.tensor_tensor(gvar[:], gmom[:, 1:2], gvar[:],
                                    OP.subtract)
            nc.vector.tensor_scalar(gvar[:], gvar[:], EPS, None, OP.add)
            grstd = mpool.tile([C, 1], F32, tag="grstd")
            nc.scalar.activation(grstd[:], gvar[:], AF.Sqrt)
            nc.vector.reciprocal(grstd[:], grstd[:])
            # scale2 = gn_w * rstd ; bias2 = gn_b - mean * scale2
            sc2 = mpool.tile([C, 1], F32, tag="sc2")
            nc.vector.tensor_tensor(sc2[:], gw[:], grstd[:], OP.mult)
            bi2 = mpool.tile([C, 1], F32, tag="bi2")
            nc.vector.tensor_tensor(bi2[:], gmom[:, 0:1], sc2[:], OP.mult)
            nc.vector.tensor_tensor(bi2[:], gb[:], bi2[:], OP.subtract)

            # ---- gate + residual ----
            gg = spool.tile([C, NPIX], F32, tag="big")  # reuse sq slot
            nc.scalar.activation(gg[:].rearrange("p (a b) -> p a b", a=NT),
                                 d_sb[:], AF.Sigmoid, bias=bi2[:],
                                 scale=sc2[:])
            nc.vector.tensor_scalar(gg[:], gg[:], 1.0, None, OP.add)
            nc.vector.tensor_tensor(gg[:], gg[:], short[:], OP.mult)
            nc.sync.dma_start(
                out_d[:], gg[:].rearrange("p (r w) -> p r w", w=W))

    nc.compile()
    return nc


def _prep_inputs(inputs):
    x = np.asarray(inputs["x"], np.float32)
    dw_w = np.asarray(inputs["dw_w"], np.float32)
    dw_b = np.asarray(inputs["dw_b"], np.float32)
    pw_w = np.asarray(inputs["pw_w"], np.float32)
    pw_b = np.asarray(inputs["pw_b"], np.float32)
    off_w = np.asarray(inputs["off_w"], np.float32)
    off_b = np.asarray(inputs["off_b"], np.float32)
    de_w = np.asarray(inputs["de_w"], np.float32)
    de_b = np.asarray(inputs["de_b"], np.float32)
    gn_w = np.asarray(inputs["gn_w"], np.float32)
    gn_b = np.asarray(inputs["gn_b"], np.float32)

    bf = ml_dtypes.bfloat16
    # fused conv1 weights: wf[t][c, o] = pw_w[o, c] * dw_w[c, 0, ty, tx]
    dwt = dw_w.reshape(C, 9)                        # [c, t]
    wf = np.ascontiguousarray(
        (pw_w.T[None, :, :] * dwt.T[:, :, None]).transpose(1, 0, 2)
    ).astype(np.float32)                            # [c, t, o]
    b1 = (pw_w @ dw_b + pw_b).astype(np.float32).reshape(C, 1)
    ow = np.ascontiguousarray(
        off_w.reshape(18, C, 9).transpose(1, 2, 0)).astype(bf)   # [c, t, 18]
    obr = np.broadcast_to(off_b[None, :], (128, 18)).astype(np.float32)
    obr = np.ascontiguousarray(obr)
    wdm = np.ascontiguousarray(
        de_w.reshape(C, C, 9).transpose(1, 2, 0)).astype(bf)     # [c, k, o]
    dbc = de_b.reshape(C, 1).astype(np.float32)
    gwc = gn_w.reshape(C, 1).astype(np.float32)
    gbc = gn_b.reshape(C, 1).astype(np.float32)
    idn = np.eye(128, dtype=bf)
    onec = np.ones((C, 1), np.float32)
    # iota: w + ZX*(8 + r + ky) + (kx + 8), k = (ky+1)*3 + (kx+1)
    wv = np.arange(96)[:, None, None]
    rv = np.arange(RPC)[None, :, None]
    kyv = (np.arange(9) // 3 - 1)[None, None, :]
    kxv = (np.arange(9) % 3 - 1)[None, None, :]
    iotc = (wv + ZX * (8 + rv + kyv) + kxv + 8).astype(np.float32)

    in_maps = []
    for core in range(NCORES):
        b = core // 2
        r0 = (core % 2) * RPC
        xp = np.zeros((C, XR, XC), np.float32)
        glo, ghi = max(0, r0 - 9), min(H, r0 + 57)
        xp[:, glo - (r0 - 9):ghi - (r0 - 9), 9:105] = x[b, :, glo:ghi, :]
        vrow = np.zeros((C, GR), bf)
        vlo, vhi = max(0, r0 - 8), min(H, r0 + 56)
        vrow[:, vlo - (r0 - 8):vhi - (r0 - 8)] = bf(1.0)
        in_maps.append({
            "x_sh": xp, "vrow": vrow, "wf": wf, "b1c": b1, "ow": ow,
            "obr": obr, "wd": wdm, "dbc": dbc, "gwc": gwc, "gbc": gbc,
            "idn": idn, "onec": onec, "iotc": iotc,
        })
    return in_maps


def get_program():
    if "nc" not in _CACHE:
        _CACHE["nc"] = _build_program()
    return _CACHE["nc"]


def kernel(**inputs):
    nc = get_program()
    in_maps = _prep_inputs(inputs)
    res = run_bass_kernel_spmd(nc, in_maps, core_ids=list(range(NCORES)))
    out = np.empty((B, C, H, W), np.float32)
    for core in range(NCORES):
        b = core // 2
        r0 = (core % 2) * RPC
        out[b, :, r0:r0 + RPC, :] = res.results[core]["out_sh"]
    return out
